# revision 28
# baseline (speedup 1.0000x reference)
"""AttentionBlock (GroupNorm + 4-head self-attention + proj + residual) on 8
Trainium2 NeuronCores.

Sharding: core i handles batch b = i // 4 and query slice s = i % 4 (1024 of
4096 query positions).  Each core computes GroupNorm + full k/v for its batch
(replicated within the 4 cores of a batch), attention for all 4 heads over its
query slice, and the output projection + residual for its slice.  Outputs are
disjoint [1024, 256] (query-major) slices; the host concatenates and
transposes back to [2, 256, 64, 64].

Fast path vs the v0 baseline:
  - softmax exp is split across ScalarE (exact exp -> fp8e4 out) and VectorE
    (Schraudolph exp: one tensor_scalar f32->u8 whose bits ARE the fp8e4
    weight; HW rounds-to-nearest and saturates at 0, giving the clamp free).
  - attn @ v runs as fp8 DoubleRow matmuls (256-key contraction per
    instruction), halving the dominant PE stream time.
  - GroupNorm stats are computed per 512-col chunk as the x DMA lands
    (tile 0 on DVE bn_stats, tile 1 on ScalarE square/identity accum);
    a single fused [16,4] group chain serves both tiles.
  - PE warmup matmuls + DMA-gated keepalives hold the HAM clock-gate open
    so qkv runs at 2.4 GHz.
  - k1/q1/v matmuls and their psum->sbuf casts are interleaved into the
    attention instruction stream (PE has slack there; exp is the wall).
"""

import sys
import time
from contextlib import ExitStack

if "/opt/trn_rl_repo" not in sys.path:
    sys.path.insert(0, "/opt/trn_rl_repo")

import numpy as np

import concourse.bacc as bacc
import concourse.tile as tile
import concourse.mybir as mybir
from concourse import bass_utils

F32 = mybir.dt.float32
F16 = mybir.dt.float16
F8 = mybir.dt.float8e4
I32 = mybir.dt.int32
U8 = mybir.dt.uint8
AF = mybir.ActivationFunctionType
ALU = mybir.AluOpType
MPM = mybir.MatmulPerfMode

C = 256  # channels
N = 4096  # h*w
NS = 1024  # query slice per core
H = 4  # heads
HD = 64  # head dim
G = 32  # groups
GS = 8  # channels per group
EPS = 1e-5
SCALE = HD**-0.5  # 0.125

# Schraudolph exp on raw scores s: fp8e4 bits = rne(clip(A8*s + B8, 0, 255))
# approximates exp(s*SCALE - 3).  (8/ln2)*SCALE = 1.4427066;
# bias 56 = 7*8 (fp8e4 exponent bias), minus the 3-shift in exponent units.
A8 = 1.4427066
B8 = 56.0 - 3.0 * (8.0 / 0.6931471805599453)

_cached = {}
DEBUG_DUMPS = False


def _build():
    nc = bacc.Bacc("TRN2", target_bir_lowering=False, debug=False, num_devices=8)

    xb_d = nc.dram_tensor("xb", [C, N], F32, kind="ExternalInput")
    xs_d = nc.dram_tensor("xs", [C, NS], F32, kind="ExternalInput")
    xst_d = nc.dram_tensor("xst", [NS, C], F32, kind="ExternalInput")
    wqkvt_d = nc.dram_tensor("wqkvt", [C, 3 * C], F32, kind="ExternalInput")
    wpt_d = nc.dram_tensor("wpt", [C, C], F32, kind="ExternalInput")
    gnw_d = nc.dram_tensor("gnw", [2, 128, 1], F32, kind="ExternalInput")
    gnb_d = nc.dram_tensor("gnb", [2, 128, 1], F32, kind="ExternalInput")
    bproj_d = nc.dram_tensor("bproj", [1, C], F32, kind="ExternalInput")
    gmap_d = nc.dram_tensor("gmap", [128, 16], F32, kind="ExternalInput")
    gmapt_d = nc.dram_tensor("gmapt", [16, 128], F32, kind="ExternalInput")
    onescol_d = nc.dram_tensor("onescol", [128, 1], F32, kind="ExternalInput")
    onesrow_d = nc.dram_tensor("onesrow", [1, 128], F32, kind="ExternalInput")
    yt_d = nc.dram_tensor("yt", [NS, C], F32, kind="ExternalOutput")
    if DEBUG_DUMPS:
        dvt_d = nc.dram_tensor("d_vt", [128, 16 * 2 * 4 * 68], U8, kind="ExternalOutput")
        dk_d = nc.dram_tensor("d_k", [2, 128, N], F16, kind="ExternalOutput")
        dq_d = nc.dram_tensor("d_q", [2, 128, NS], F16, kind="ExternalOutput")
        du_d = nc.dram_tensor("d_u", [2, 128, NS], F16, kind="ExternalOutput")
        dhn_d = nc.dram_tensor("d_hn", [2, 128, N], F16, kind="ExternalOutput")
        du2_d = nc.dram_tensor("d_u2", [2, 65, 512], F32, kind="ExternalOutput")
        drb_d = nc.dram_tensor("d_rb", [2, 64, 512], F32, kind="ExternalOutput")
        dp8_d = nc.dram_tensor("d_p8", [128, 2048], U8, kind="ExternalOutput")

    xb = xb_d.ap()
    yt = yt_d.ap()

    # exp engine assignment: True -> ScalarE, False -> DVE Schraudolph.
    # 128 exp tiles total; tune the ratio from the trace.
    def exp_on_act(idx):
        return idx % 2 == 0

    with tile.TileContext(nc) as tc:
        with (
            tc.tile_pool(name="const", bufs=1) as constp,
            tc.tile_pool(name="main", bufs=1) as mainp,
            tc.tile_pool(name="rot", bufs=3) as rotp,
        ):
            # ---- persistent tiles ------------------------------------
            gmap = constp.tile([128, 16], F32, tag="gmap", name="gmap")
            gmapt = constp.tile([16, 128], F32, tag="gmapt", name="gmapt")
            onescol = constp.tile([128, 1], F32, tag="onescol", name="onescol")
            onesrow = constp.tile([1, 128], F32, tag="onesrow", name="onesrow")
            onesrow16 = constp.tile([1, 128], F16, tag="onesrow16", name="onesrow16")
            ones8 = constp.tile([128, 1], F8, tag="ones8", name="ones8")
            bproj_f = constp.tile([1, C], F32, tag="bproj_f", name="bproj_f")
            biasm3 = constp.tile([128, 1], F32, tag="biasm3", name="biasm3")
            gnw = [constp.tile([128, 1], F32, tag=f"gnw{t}", name=f"gnw{t}") for t in range(2)]
            gnb = [constp.tile([128, 1], F32, tag=f"gnb{t}", name=f"gnb{t}") for t in range(2)]
            wp_r = []

            k_sb = [mainp.tile([128, N], F16, tag=f"k{t}", name=f"k{t}") for t in range(2)]
            q_sb = [mainp.tile([128, NS], F16, tag=f"q{t}", name=f"q{t}") for t in range(2)]
            # v^T fp8, DoubleRow interleaved: [128, c(16), i(2), h(4), m(68)]
            # (m: 64 head-dim cols + ones col at 64; pitch 68 for 16B align)
            vt = mainp.tile([128, 16 * 2 * 4 * 68], F8, tag="vt", name="vt")
            vt_5d = vt[:].rearrange("p (c i h m) -> p c i h m", c=16, i=2, h=4)
            u_sb = [mainp.tile([128, NS], F16, tag=f"u{t}", name=f"u{t}") for t in range(2)]
            # f16 copies of raw x (GN affine is folded into the qkv weights)
            xh = [mainp.tile([128, N], F16, tag=f"xh{t}", name=f"xh{t}") for t in range(2)]
            xsh = [mainp.tile([128, NS], F16, tag=f"xsh{t}", name=f"xsh{t}") for t in range(2)]
            xst = mainp.tile([128, 8 * C], F32, tag="xst", name="xst")
            wq_a16 = [
                constp.tile([128, 3 * C], F16, tag=f"wqa{t}", name=f"wqa{t}")
                for t in range(2)
            ]
            beta_k = [constp.tile([128, 1], F32, tag=f"bk{t}", name=f"bk{t}") for t in range(2)]
            beta_q = [constp.tile([128, 1], F32, tag=f"bq{t}", name=f"bq{t}") for t in range(2)]
            bproj_e16 = constp.tile([1, C], F16, tag="bpe16", name="bpe16")

            with (
                tc.tile_pool(name="stage", bufs=1) as stagep,
                tc.tile_pool(name="wps", bufs=1, space="PSUM") as wpsp,
                tc.tile_pool(name="psgn", bufs=1, space="PSUM") as psgn,
            ):
                # ---- PE warmup (HAM clock-gate) ----------------------
                junk16 = stagep.tile([128, 512], F16, tag="junk", name="junk")
                nc.vector.memset(junk16[:], 0.5)
                wps = wpsp.tile([128, 512], F32, tag="w", name="w")
                for r in range(16):
                    nc.tensor.matmul(
                        wps[:], junk16[:, 0:128], junk16[:], start=True, stop=True
                    )

                # prefetch the exp activation table set
                dummy = stagep.tile([1, 1], F32, tag="dummy", name="dummy")
                nc.vector.memset(dummy[:], 1.0)
                nc.scalar.activation(dummy[:], dummy[:], AF.Exp)
                nc.vector.memset(biasm3[:], -3.0)

                # ---- constants + weights + xs on gpsimd SWDGE --------
                nc.gpsimd.dma_start(gmap[:], gmap_d.ap())
                nc.gpsimd.dma_start(gmapt[:], gmapt_d.ap())
                nc.gpsimd.dma_start(onescol[:], onescol_d.ap())
                nc.gpsimd.dma_start(onesrow[:], onesrow_d.ap())
                for t in range(2):
                    nc.gpsimd.dma_start(gnw[t][:], gnw_d.ap()[t])
                    nc.gpsimd.dma_start(gnb[t][:], gnb_d.ap()[t])
                nc.gpsimd.dma_start(bproj_f[:], bproj_d.ap())
                wqf = []
                wpf = []
                for t in range(2):
                    w = stagep.tile([128, 3 * C], F32, tag=f"wqf{t}", name=f"wqf{t}")
                    nc.gpsimd.dma_start(w[:], wqkvt_d.ap()[t * 128 : t * 128 + 128, :])
                    wqf.append(w)
                    wp = stagep.tile([128, C], F32, tag=f"wpf{t}", name=f"wpf{t}")
                    nc.gpsimd.dma_start(wp[:], wpt_d.ap()[t * 128 : t * 128 + 128, :])
                    wpf.append(wp)
                xs_sb = [
                    stagep.tile([128, NS], F32, tag=f"xs{t}", name=f"xs{t}")
                    for t in range(2)
                ]
                for t in range(2):
                    nc.gpsimd.dma_start(
                        xs_sb[t][:], xs_d.ap()[t * 128 : t * 128 + 128, :]
                    )
                # residual slice (needed only at proj time)
                nc.gpsimd.dma_start(
                    xst[:].rearrange("p (a f) -> p a f", a=8),
                    xst_d.ap().rearrange("(a p) f -> p a f", p=128),
                )

                nc.vector.tensor_copy(onesrow16[:], onesrow[:])
                nc.vector.tensor_copy(ones8[:], onescol[:])
                # ones columns of v^T (softmax denominator accumulators)
                nc.vector.tensor_copy(
                    vt_5d[:, :, :, :, 64:65],
                    ones8[:].to_broadcast([128, 16, 2, 4, 1]),
                )
                for t in range(2):
                    wpr = constp.tile([128, C], F16, tag=f"wpr{t}", name=f"wpr{t}")
                    nc.gpsimd.tensor_copy(wpr[:], wpf[t][:])
                    wp_r.append(wpr)

                # ---- x in 2048-col halves: tile0 via sync, tile1 via
                # scalar hwdge (separate queues run concurrently) ------
                x_sb = [
                    stagep.tile([128, N], F32, tag=f"x{t}", name=f"x{t}")
                    for t in range(2)
                ]
                bno = stagep.tile([128, 48], F32, tag="bno0", name="bno0")
                sxs = stagep.tile([128, 4], F32, tag="sxs", name="sxs")
                scr = stagep.tile([128, 2048], F16, tag="scr", name="scr")
                for hf in range(2):
                    hsl = slice(hf * 2048, hf * 2048 + 2048)
                    nc.sync.dma_start(x_sb[0][:, hsl], xb[0:128, hsl])
                    nc.scalar.dma_start(
                        x_sb[1][:, hsl], xb[128:256, hsl]
                    )
                    for t in range(2):
                        # keepalive matmul (reads the half; keeps HAM warm)
                        nc.tensor.matmul(
                            wps[0:1, 0:64], onescol[:],
                            x_sb[t][:, hf * 2048 : hf * 2048 + 64],
                            start=True, stop=True,
                        )
                    # tile0 stats on DVE + f16 cast
                    for j in range(4):
                        ch = 4 * hf + j
                        nc.vector.bn_stats(
                            bno[:, ch * 6 : ch * 6 + 6],
                            x_sb[0][:, ch * 512 : ch * 512 + 512],
                        )
                    nc.vector.tensor_copy(xh[0][:, hsl], x_sb[0][:, hsl])
                    # tile1 stats on ScalarE + f16 cast on gpsimd
                    nc.scalar.activation(
                        scr[:], x_sb[1][:, hsl], AF.Identity,
                        accum_out=sxs[:, hf : hf + 1],
                    )
                    nc.scalar.activation(
                        scr[:], x_sb[1][:, hsl], AF.Square,
                        accum_out=sxs[:, 2 + hf : 3 + hf],
                    )
                    nc.gpsimd.tensor_copy(xh[1][:, hsl], x_sb[1][:, hsl])
                for t in range(2):
                    nc.gpsimd.tensor_copy(xsh[t][:], xs_sb[t][:])

                # sm = [mean0, E[x^2]0, mean1, E[x^2]1]  [128, 4]
                sm = stagep.tile([128, 4], F32, tag="sm", name="sm")
                agg = stagep.tile([128, 2], F32, tag="agg0", name="agg0")
                nc.vector.bn_aggr(agg[:], bno[:].rearrange("p (c s) -> p c s", c=16))
                nc.vector.tensor_copy(sm[:, 0:1], agg[:, 0:1])
                msq = stagep.tile([128, 1], F32, tag="msq0", name="msq0")
                nc.vector.tensor_tensor(msq[:], agg[:, 0:1], agg[:, 0:1], op=ALU.mult)
                nc.vector.tensor_tensor(sm[:, 1:2], agg[:, 1:2], msq[:], op=ALU.add)
                ssum = stagep.tile([128, 2], F32, tag="ssum", name="ssum")
                nc.vector.tensor_reduce(
                    ssum[:], sxs[:].rearrange("p (a c) -> p a c", a=2),
                    axis=mybir.AxisListType.X, op=ALU.add,
                )
                nc.vector.tensor_scalar(
                    sm[:, 2:4], ssum[:], 1.0 / N, None, op0=ALU.mult
                )

                # ---- fused group chain on [16, 4] --------------------
                gp = psgn.tile([16, 4], F32, tag="gp", name="gp")
                nc.tensor.matmul(gp[:], gmap[:], sm[:], start=True, stop=True)
                grs = stagep.tile([16, 4], F32, tag="grs", name="grs")
                nc.vector.tensor_scalar(grs[:], gp[:], 1.0 / GS, None, op0=ALU.mult)
                mu = grs[:].rearrange("p (g s) -> p g s", g=2)[:, :, 0]
                e2 = grs[:].rearrange("p (g s) -> p g s", g=2)[:, :, 1]
                mu2 = stagep.tile([16, 2], F32, tag="mu2", name="mu2")
                nc.vector.tensor_tensor(mu2[:], mu, mu, op=ALU.mult)
                vg = stagep.tile([16, 2], F32, tag="vg", name="vg")
                nc.vector.tensor_tensor(vg[:], e2, mu2[:], op=ALU.subtract)
                ve = stagep.tile([16, 2], F32, tag="ve", name="ve")
                nc.vector.tensor_scalar(ve[:], vg[:], EPS, None, op0=ALU.add)
                mgt = stagep.tile([16, 2], I32, tag="mg", name="mg")
                nc.vector.memset(mgt[:], 0x5F3759DF)
                half = stagep.tile([16, 2], I32, tag="hf", name="hf")
                nc.vector.tensor_scalar(
                    half[:], ve[:].bitcast(I32), 1, None,
                    op0=ALU.logical_shift_right,
                )
                y = stagep.tile([16, 2], F32, tag="qy", name="qy")
                nc.vector.tensor_tensor(
                    y[:].bitcast(I32), mgt[:], half[:], op=ALU.subtract
                )
                for it in range(2):
                    ysq = stagep.tile([16, 2], F32, tag=f"ys{it}", name=f"ys{it}")
                    nc.vector.tensor_tensor(ysq[:], y[:], y[:], op=ALU.mult)
                    vy2 = stagep.tile([16, 2], F32, tag=f"vy{it}", name=f"vy{it}")
                    nc.vector.tensor_tensor(vy2[:], ysq[:], ve[:], op=ALU.mult)
                    hh = stagep.tile([16, 2], F32, tag=f"hh{it}", name=f"hh{it}")
                    nc.vector.tensor_scalar(
                        hh[:], vy2[:], -0.5, 1.5, op0=ALU.mult, op1=ALU.add
                    )
                    if it == 0:
                        yn = stagep.tile([16, 2], F32, tag="yn", name="yn")
                        nc.vector.tensor_tensor(yn[:], y[:], hh[:], op=ALU.mult)
                        y = yn
                    else:
                        # write rstd into grs cols {1, 3}
                        nc.vector.tensor_tensor(e2, y[:], hh[:], op=ALU.mult)

                # ---- per-channel a, b --------------------------------
                a_t = []
                b_t = []
                for t in range(2):
                    bp = psgn.tile([128, 2], F32, tag="bp", name="bp")
                    nc.tensor.matmul(
                        bp[:], gmapt[:], grs[:, 2 * t : 2 * t + 2],
                        start=True, stop=True,
                    )
                    ab = stagep.tile([128, 2], F32, tag=f"ab{t}", name=f"ab{t}")
                    nc.vector.tensor_copy(ab[:], bp[:])
                    av = stagep.tile([128, 1], F32, tag=f"av{t}", name=f"av{t}")
                    nc.vector.tensor_tensor(av[:], ab[:, 1:2], gnw[t][:], op=ALU.mult)
                    tmp = stagep.tile([128, 1], F32, tag=f"tmp{t}", name=f"tmp{t}")
                    nc.vector.tensor_tensor(tmp[:], ab[:, 0:1], av[:], op=ALU.mult)
                    bv = stagep.tile([128, 1], F32, tag=f"bv{t}", name=f"bv{t}")
                    nc.vector.tensor_tensor(bv[:], gnb[t][:], tmp[:], op=ALU.subtract)
                    a_t.append(av)
                    b_t.append(bv)

                # ---- fold GN affine into qkv weights -----------------
                # k = (wk*a) @ x + (wk @ b) x 1;  same for q.  The v bias
                # passes through softmax (weights sum to 1) and folds into
                # the projection bias: bproj_eff = bproj + wp @ (wv @ b).
                for ct in range(2):
                    nc.vector.tensor_scalar(
                        wq_a16[ct][:], wqf[ct][:], a_t[ct][:], None, op0=ALU.mult
                    )
                for mt in range(2):
                    for dst, off in ((beta_q[mt], mt * 128),
                                     (beta_k[mt], C + mt * 128)):
                        b_ps = psgn.tile([128, 1], F32, tag="bps", name="bps")
                        for ct in range(2):
                            nc.tensor.matmul(
                                b_ps[:], wqf[ct][:, off : off + 128], b_t[ct][:],
                                start=(ct == 0), stop=(ct == 1),
                            )
                        nc.vector.tensor_copy(dst[:], b_ps[:])
                bv_sb = []
                for ot in range(2):
                    b_ps = psgn.tile([128, 1], F32, tag="bps", name="bps")
                    for ct in range(2):
                        nc.tensor.matmul(
                            b_ps[:],
                            wqf[ct][:, 2 * C + ot * 128 : 2 * C + ot * 128 + 128],
                            b_t[ct][:],
                            start=(ct == 0), stop=(ct == 1),
                        )
                    bv = stagep.tile([128, 1], F32, tag=f"bvv{ot}", name=f"bvv{ot}")
                    nc.vector.tensor_copy(bv[:], b_ps[:])
                    bv_sb.append(bv)
                row_ps = psgn.tile([1, C], F32, tag="row", name="row")
                for ot in range(2):
                    nc.tensor.matmul(
                        row_ps[:], bv_sb[ot][:], wpf[ot][:],
                        start=(ot == 0), stop=(ot == 1),
                    )
                bpe = stagep.tile([1, C], F32, tag="bpe", name="bpe")
                nc.vector.tensor_tensor(bpe[:], bproj_f[:], row_ps[:], op=ALU.add)
                nc.vector.tensor_copy(bproj_e16[:], bpe[:])

            # ---- qkv + attention + proj ------------------------------
            exp_idx = [0]
            cast_idx = [0]

            with (
                tc.tile_pool(name="pss", bufs=1, space="PSUM") as pss,
                tc.tile_pool(name="psu", bufs=1, space="PSUM") as psu,
            ):
                def stile():
                    return pss.tile([128, 1024], F32, tag="s", name="s", bufs=3)

                def cast(dst, src, bias=None, force=None):
                    """psum -> sbuf cast (+ per-partition bias) on ACT/DVE."""
                    eng = force if force is not None else (
                        "act" if cast_idx[0] % 2 == 0 else "dve"
                    )
                    cast_idx[0] += 1
                    if eng == "act":
                        if bias is None:
                            nc.scalar.activation(dst, src, AF.Copy)
                        else:
                            nc.scalar.activation(
                                dst, src, AF.Identity, bias=bias
                            )
                    else:
                        if bias is None:
                            nc.vector.tensor_copy(dst, src)
                        else:
                            nc.vector.tensor_scalar(
                                dst, src, bias, None, op0=ALU.add
                            )

                def emit_k_pair(mt, pair, force=None):
                    # k columns [pair*1024, (pair+1)*1024) -> one s tile
                    ps = stile()
                    for half in range(2):
                        csl = slice(pair * 1024 + half * 512,
                                    pair * 1024 + half * 512 + 512)
                        for ct in range(2):
                            nc.tensor.matmul(
                                ps[:, half * 512 : half * 512 + 512],
                                wq_a16[ct][:, C + mt * 128 : C + mt * 128 + 128],
                                xh[ct][:, csl],
                                start=(ct == 0),
                                stop=(ct == 1),
                            )
                    cast(k_sb[mt][:, pair * 1024 : pair * 1024 + 1024], ps[:],
                         bias=beta_k[mt][:], force=force)

                def emit_q_both(nch, force=None):
                    # q for both head-pair tiles, one 512-query slice
                    qsl = slice(nch * 512, nch * 512 + 512)
                    ps = stile()
                    for mt in range(2):
                        for ct in range(2):
                            nc.tensor.matmul(
                                ps[:, mt * 512 : mt * 512 + 512],
                                wq_a16[ct][:, mt * 128 : mt * 128 + 128],
                                xsh[ct][:, qsl],
                                start=(ct == 0),
                                stop=(ct == 1),
                            )
                    for mt in range(2):
                        cast(q_sb[mt][:, qsl], ps[:, mt * 512 : mt * 512 + 512],
                             bias=beta_q[mt][:], force=force)

                def emit_v_quad(cp, force=None):
                    # v^T for key chunks 4cp..4cp+3 -> one s tile
                    ps = stile()
                    for ii in range(4):
                        mch = 4 * cp + ii
                        for ct in range(2):
                            nc.tensor.matmul(
                                ps[:, ii * 256 : ii * 256 + 256],
                                xh[ct][:, mch * 128 : mch * 128 + 128],
                                wq_a16[ct][:, 2 * C : 3 * C],
                                start=(ct == 0),
                                stop=(ct == 1),
                            )
                    for half in range(2):
                        cast(
                            vt_5d[:, 2 * cp + half, :, :, 0:64],
                            ps[:, half * 512 : half * 512 + 512].rearrange(
                                "p (i h m) -> p i h m", i=2, h=4
                            ),
                            force=force,
                        )

                def emit_exp(s_ps, p8p, i):
                    in_view = s_ps[:].rearrange("p (h n) -> p h n", h=2)
                    out_view = p8p[:].rearrange(
                        "p (h i n) -> p h i n", h=2, i=2
                    )[:, :, i, :]
                    if exp_on_act(exp_idx[0]):
                        nc.scalar.activation(
                            out_view, in_view, AF.Exp, bias=biasm3[:], scale=SCALE
                        )
                    else:
                        nc.vector.tensor_scalar(
                            out_view.bitcast(U8), in_view, A8, B8,
                            op0=ALU.mult, op1=ALU.add,
                        )
                    exp_idx[0] += 1

                def attention_block(nch, hp, extra):
                    """extra: list of callbacks to interleave, one per c.
                    AV matmuls trail the scores by 2 c-groups so the PE
                    (in-order queue) never stalls on exp completion."""
                    qsl = slice(nch * 512, nch * 512 + 512)
                    u2 = [
                        psu.tile([65, 512], F32, tag=f"u{j}", name=f"u{j}", bufs=1)
                        for j in range(2)
                    ]

                    def emit_av(c, p8p):
                        for j in range(2):
                            nc.tensor.matmul(
                                u2[j][:],
                                vt_5d[:, c, :, 2 * hp + j, 0:65],
                                p8p[:].rearrange(
                                    "p (h i n) -> p h i n", h=2, i=2
                                )[:, j, :, :],
                                start=(c == 0),
                                stop=(c == 15),
                                perf_mode=MPM.DoubleRow,
                            )

                    pend = []
                    for c in range(16):
                        if c < len(extra):
                            extra[c]()
                        p8p = rotp.tile(
                            [128, 2048], F8, tag="p8", name="p8", bufs=4
                        )
                        for i in range(2):
                            mch = 2 * c + i
                            msl = slice(mch * 128, mch * 128 + 128)
                            s_ps = stile()
                            nc.tensor.matmul(
                                s_ps[:, 0:512],
                                k_sb[hp][0:64, msl],
                                q_sb[hp][0:64, qsl],
                                start=True, stop=True,
                            )
                            nc.tensor.matmul(
                                s_ps[:, 512:1024],
                                k_sb[hp][64:128, msl],
                                q_sb[hp][64:128, qsl],
                                start=True, stop=True,
                            )
                            emit_exp(s_ps, p8p, i)
                        pend.append((c, p8p))
                        if DEBUG_DUMPS and nch == 0 and hp == 0 and c == 0:
                            nc.sync.dma_start(dp8_d.ap(), p8p[:].bitcast(U8))
                        if len(pend) > 2:
                            emit_av(*pend.pop(0))
                    for item in pend:
                        emit_av(*item)
                    # normalize: u_sb[hp][64j:64j+64, qsl] = u2[0:64] / u2[64]
                    for j in range(2):
                        lh = rotp.tile([1, 512], F32, tag="lh", name="lh")
                        nc.vector.tensor_copy(lh[:], u2[j][64:65, :])
                        rh = rotp.tile([1, 512], F32, tag="rh", name="rh")
                        nc.vector.reciprocal_approx_fast(rh[:], lh[:])
                        rb = rotp.tile([64, 512], F32, tag="rb", name="rb")
                        nc.gpsimd.partition_broadcast(rb[:], rh[:])
                        nc.vector.tensor_tensor(
                            u_sb[hp][64 * j : 64 * j + 64, qsl],
                            u2[j][0:64, :],
                            rb[:],
                            op=ALU.mult,
                        )
                        if DEBUG_DUMPS and nch == 0 and hp == 0:
                            u2c = rotp.tile([65, 512], F32, tag="u2c", name="u2c")
                            nc.vector.tensor_copy(u2c[:], u2[j][:])
                            nc.sync.dma_start(du2_d.ap()[j], u2c[:])
                            nc.sync.dma_start(drb_d.ap()[j], rb[:])

                def emit_proj(nc8):
                    sl = slice(nc8 * 128, nc8 * 128 + 128)
                    ps = stile()
                    y_ps = ps[:, 0:256]
                    for ct in range(2):
                        nc.tensor.matmul(
                            y_ps,
                            u_sb[ct][:, sl],
                            wp_r[ct][:],
                            start=(ct == 0),
                            stop=False,
                        )
                    nc.tensor.matmul(
                        y_ps, onesrow16[:], bproj_e16[:], start=False, stop=True
                    )
                    out_t = rotp.tile([128, C], F32, tag="out", name="out")
                    nc.vector.tensor_tensor(
                        out_t[:], y_ps, xst[:, nc8 * C : nc8 * C + C], op=ALU.add
                    )
                    nc.sync.dma_start(yt[sl, :], out_t[:])

                # strictly-needed prologue: k tile 0, q (nch 0); casts on
                # ACT (DVE is still producing hn at this point)
                for pair in range(4):
                    emit_k_pair(0, pair, force="act")
                emit_q_both(0, force="act")

                # block (nch0, hp0): interleave v quads + q-nch1
                extra0 = [lambda cp=cp: emit_v_quad(cp) for cp in range(8)]
                extra0.append(lambda: emit_q_both(1))
                attention_block(0, 0, extra0)

                # block (nch0, hp1): interleave k tile 1
                extra1 = [lambda p=p: emit_k_pair(1, p) for p in range(4)]
                attention_block(0, 1, extra1)

                # block (nch1, hp0): interleave proj of nch0
                extra2 = [lambda n=n: emit_proj(n) for n in range(4)]
                attention_block(1, 0, extra2)
                attention_block(1, 1, [])
                for nc8 in range(4, 8):
                    emit_proj(nc8)

                if DEBUG_DUMPS:
                    nc.sync.dma_start(dvt_d.ap(), vt[:].bitcast(U8))
                    for t in range(2):
                        nc.sync.dma_start(dk_d.ap()[t], k_sb[t][:])
                        nc.sync.dma_start(dq_d.ap()[t], q_sb[t][:])
                        nc.sync.dma_start(du_d.ap()[t], u_sb[t][:])
                        nc.sync.dma_start(dhn_d.ap()[t], xh[t][:])

    nc.compile()
    return nc


def _in_maps(inputs):
    x = np.ascontiguousarray(np.asarray(inputs["x"], dtype=np.float32))
    gn_scale = np.asarray(inputs["gn_scale"], dtype=np.float32)
    gn_bias = np.asarray(inputs["gn_bias"], dtype=np.float32)
    w_qkv = np.asarray(inputs["w_qkv"], dtype=np.float32)
    w_proj = np.asarray(inputs["w_proj"], dtype=np.float32)
    b_proj = np.asarray(inputs["b_proj"], dtype=np.float32)

    B = x.shape[0]
    xf = x.reshape(B, C, N)
    wqkvt = np.ascontiguousarray(w_qkv.T)
    wpt = np.ascontiguousarray(w_proj.T)
    gnw = np.ascontiguousarray(gn_scale.reshape(2, 128, 1))
    gnb = np.ascontiguousarray(gn_bias.reshape(2, 128, 1))
    bproj = np.ascontiguousarray(b_proj.reshape(1, C))
    gmap = np.zeros((128, 16), dtype=np.float32)
    gmap[np.arange(128), np.arange(128) // GS] = 1.0
    gmapt = np.ascontiguousarray(gmap.T)
    onescol = np.ones((128, 1), dtype=np.float32)
    onesrow = np.ones((1, 128), dtype=np.float32)

    maps = []
    for core in range(8):
        b, s = core // 4, core % 4
        xs = np.ascontiguousarray(xf[b][:, s * NS : (s + 1) * NS])
        maps.append(
            {
                "xb": xf[b],
                "xs": xs,
                "xst": np.ascontiguousarray(xs.T),
                "wqkvt": wqkvt,
                "wpt": wpt,
                "gnw": gnw,
                "gnb": gnb,
                "bproj": bproj,
                "gmap": gmap,
                "gmapt": gmapt,
                "onescol": onescol,
                "onesrow": onesrow,
            }
        )
    return maps


def _run(inputs, trace=False):
    if "nc" not in _cached:
        _cached["nc"] = _build()
    nc = _cached["nc"]
    maps = _in_maps(inputs)
    res = None
    for attempt in range(4):
        try:
            res = bass_utils.run_bass_kernel_spmd(
                nc, maps, core_ids=list(range(8)), trace=trace
            )
            break
        except Exception:
            if attempt == 3:
                raise
            time.sleep(10.0 * (attempt + 1))
    outs = np.stack([res.results[c]["yt"] for c in range(8)])  # [8, NS, C]
    y = outs.reshape(2, 4 * NS, C).transpose(0, 2, 1).reshape(2, C, 64, 64)
    return np.ascontiguousarray(y.astype(np.float32)), res


def kernel(**inputs):
    y, _ = _run(inputs, trace=False)
    return y


# revision 29
# speedup vs baseline: 1.2375x; 1.2375x over previous
"""AttentionBlock (GroupNorm + 4-head self-attention + proj + residual) on 8
Trainium2 NeuronCores.

Sharding: core i handles batch b = i // 4 and query slice s = i % 4 (1024 of
4096 query positions).  Each core computes full k/v for its batch (replicated
within the 4 cores of a batch), attention for all 4 heads over its query
slice, and the output projection + residual for its slice.  Outputs are
disjoint [1024, 256] (query-major) slices; the host concatenates and
transposes back to [2, 256, 64, 64].

Key design points:
  - Host sends x / weights / residual in f16 and ROTATES each core's x so
    its query slice sits at columns 0:1024 (key order is attention-
    invariant): 2.7 MB of demand-critical DMA instead of 6.2 MB.
  - The GroupNorm affine folds into the qkv weights: k = (wk*a) @ x + bk x 1
    (bk via tiny matmuls of b against the weights); the v bias passes
    through softmax (weights sum to 1) and folds into the projection bias.
    So qkv matmuls consume raw f16 x chunks as they land.
  - softmax exp splits across ScalarE (exact exp -> fp8e4) and VectorE
    (Schraudolph exp: one tensor_scalar f32->u8 whose bits ARE the fp8e4
    weight; HW rounds-to-nearest and saturates at 0 = free clamp).
  - attn @ v runs as fp8 DoubleRow matmuls (256-key contraction), halving
    the dominant PE stream time; a ones column in v^T accumulates the
    softmax denominators.
  - The PE queue is in-order, so AV matmuls trail their scores by 2 chunk
    groups, v/k1/q1/proj work is interleaved into the attention stream,
    and warmup + DMA-gated keepalive matmuls hold the HAM clock-gate open.
"""

import sys
import time

if "/opt/trn_rl_repo" not in sys.path:
    sys.path.insert(0, "/opt/trn_rl_repo")

import numpy as np

import concourse.bacc as bacc
import concourse.tile as tile
import concourse.mybir as mybir
from concourse import bass_utils

F32 = mybir.dt.float32
F16 = mybir.dt.float16
F8 = mybir.dt.float8e4
I32 = mybir.dt.int32
U8 = mybir.dt.uint8
AF = mybir.ActivationFunctionType
ALU = mybir.AluOpType
MPM = mybir.MatmulPerfMode

C = 256  # channels
N = 4096  # h*w
NS = 1024  # query slice per core
H = 4  # heads
HD = 64  # head dim
G = 32  # groups
GS = 8  # channels per group
EPS = 1e-5
SCALE = HD**-0.5  # 0.125

# Schraudolph exp on raw scores s: fp8e4 bits = rne(clip(A8*s + B8, 0, 255))
# approximates exp(s*SCALE - 3).  (8/ln2)*SCALE = 1.4427066;
# bias 56 = 7*8 (fp8e4 exponent bias), minus the 3-shift in exponent units.
A8 = 1.4427066
B8 = 56.0 - 3.0 * (8.0 / 0.6931471805599453)

_cached = {}
DEBUG_DUMPS = False


def _build():
    nc = bacc.Bacc("TRN2", target_bir_lowering=False, debug=False, num_devices=8)

    xb_d = nc.dram_tensor("xb", [C, N], F16, kind="ExternalInput")
    xst_d = nc.dram_tensor("xst", [NS, C], F16, kind="ExternalInput")
    wqkvt_d = nc.dram_tensor("wqkvt", [C, 3 * C], F16, kind="ExternalInput")
    wpt_d = nc.dram_tensor("wpt", [C, C], F16, kind="ExternalInput")
    gnw_d = nc.dram_tensor("gnw", [2, 128, 1], F32, kind="ExternalInput")
    gnb_d = nc.dram_tensor("gnb", [2, 128, 1], F32, kind="ExternalInput")
    bproj_d = nc.dram_tensor("bproj", [1, C], F32, kind="ExternalInput")
    gmap_d = nc.dram_tensor("gmap", [128, 16], F32, kind="ExternalInput")
    gmapt_d = nc.dram_tensor("gmapt", [16, 128], F32, kind="ExternalInput")
    onescol_d = nc.dram_tensor("onescol", [128, 1], F32, kind="ExternalInput")
    onesrow_d = nc.dram_tensor("onesrow", [1, 128], F32, kind="ExternalInput")
    yt_d = nc.dram_tensor("yt", [NS, C], F32, kind="ExternalOutput")
    if DEBUG_DUMPS:
        dvt_d = nc.dram_tensor("d_vt", [128, 16 * 2 * 4 * 68], U8, kind="ExternalOutput")
        dk_d = nc.dram_tensor("d_k", [2, 128, N], F16, kind="ExternalOutput")
        dq_d = nc.dram_tensor("d_q", [2, 128, NS], F16, kind="ExternalOutput")
        du_d = nc.dram_tensor("d_u", [2, 128, NS], F16, kind="ExternalOutput")
        du2_d = nc.dram_tensor("d_u2", [2, 65, 512], F32, kind="ExternalOutput")
        drb_d = nc.dram_tensor("d_rb", [2, 64, 512], F32, kind="ExternalOutput")
        dp8_d = nc.dram_tensor("d_p8", [128, 2048], U8, kind="ExternalOutput")

    xb = xb_d.ap()
    yt = yt_d.ap()

    # exp engine assignment: True -> ScalarE (exact), False -> DVE
    # Schraudolph.  ~9/16 of tiles on ACT (DVE also carries normalize +
    # residual + a share of the qkv casts).
    def exp_on_act(idx):
        return idx % 16 < 9

    with tile.TileContext(nc) as tc:
        with (
            tc.tile_pool(name="const", bufs=1) as constp,
            tc.tile_pool(name="main", bufs=1) as mainp,
            tc.tile_pool(name="rot", bufs=3) as rotp,
        ):
            # ---- persistent tiles ------------------------------------
            gmap = constp.tile([128, 16], F32, tag="gmap", name="gmap")
            gmapt = constp.tile([16, 128], F32, tag="gmapt", name="gmapt")
            onescol = constp.tile([128, 1], F32, tag="onescol", name="onescol")
            ones16 = constp.tile([128, 1], F16, tag="ones16", name="ones16")
            onesrow = constp.tile([1, 128], F32, tag="onesrow", name="onesrow")
            onesrow16 = constp.tile([1, 128], F16, tag="onesrow16", name="onesrow16")
            ones8 = constp.tile([128, 1], F8, tag="ones8", name="ones8")
            bproj_f = constp.tile([1, C], F32, tag="bproj_f", name="bproj_f")
            biasm3 = constp.tile([128, 1], F32, tag="biasm3", name="biasm3")
            gnw = [constp.tile([128, 1], F32, tag=f"gnw{t}", name=f"gnw{t}") for t in range(2)]
            gnb = [constp.tile([128, 1], F32, tag=f"gnb{t}", name=f"gnb{t}") for t in range(2)]

            # x in f16, per-core rotated (queries at cols 0:1024)
            x_sb = [mainp.tile([128, N], F16, tag=f"x{t}", name=f"x{t}") for t in range(2)]
            k_sb = [mainp.tile([128, N], F16, tag=f"k{t}", name=f"k{t}") for t in range(2)]
            q_sb = [mainp.tile([128, NS], F16, tag=f"q{t}", name=f"q{t}") for t in range(2)]
            # v^T fp8, DoubleRow interleaved: [128, c(16), i(2), h(4), m(68)]
            # (m: 64 head-dim cols + ones col at 64; pitch 68 for 16B align)
            vt = mainp.tile([128, 16 * 2 * 4 * 68], F8, tag="vt", name="vt")
            vt_5d = vt[:].rearrange("p (c i h m) -> p c i h m", c=16, i=2, h=4)
            u_sb = [mainp.tile([128, NS], F16, tag=f"u{t}", name=f"u{t}") for t in range(2)]
            xst = mainp.tile([128, 8 * C], F16, tag="xst", name="xst")
            wq16 = [
                mainp.tile([128, 3 * C], F16, tag=f"wq{t}", name=f"wq{t}")
                for t in range(2)
            ]
            wp_r = [
                mainp.tile([128, C], F16, tag=f"wp{t}", name=f"wp{t}")
                for t in range(2)
            ]
            wq_a16 = [
                constp.tile([128, 3 * C], F16, tag=f"wqa{t}", name=f"wqa{t}")
                for t in range(2)
            ]
            beta_k = [constp.tile([128, 1], F32, tag=f"bk{t}", name=f"bk{t}") for t in range(2)]
            beta_q = [constp.tile([128, 1], F32, tag=f"bq{t}", name=f"bq{t}") for t in range(2)]
            bproj_e16 = constp.tile([1, C], F16, tag="bpe16", name="bpe16")

            with (
                tc.tile_pool(name="stage", bufs=1) as stagep,
                tc.tile_pool(name="wps", bufs=1, space="PSUM") as wpsp,
                tc.tile_pool(name="psgn", bufs=1, space="PSUM") as psgn,
            ):
                # ---- PE warmup (HAM clock-gate) ----------------------
                junk16 = stagep.tile([128, 512], F16, tag="junk", name="junk")
                nc.vector.memset(junk16[:], 0.5)
                wps = wpsp.tile([128, 512], F32, tag="w", name="w")
                for r in range(16):
                    nc.tensor.matmul(
                        wps[:], junk16[:, 0:128], junk16[:], start=True, stop=True
                    )

                # prefetch the exp activation table set
                dummy = stagep.tile([1, 1], F32, tag="dummy", name="dummy")
                nc.vector.memset(dummy[:], 1.0)
                nc.scalar.activation(dummy[:], dummy[:], AF.Exp)
                nc.vector.memset(biasm3[:], -3.0)
                nc.vector.memset(ones16[:], 1.0)

                # ---- consts + weights on gpsimd SWDGE; x halves on the
                # two hwdge queues (fabric is shared; fewest bytes wins) -
                nc.gpsimd.dma_start(gmap[:], gmap_d.ap())
                nc.gpsimd.dma_start(gmapt[:], gmapt_d.ap())
                nc.gpsimd.dma_start(onescol[:], onescol_d.ap())
                nc.gpsimd.dma_start(onesrow[:], onesrow_d.ap())
                for t in range(2):
                    nc.gpsimd.dma_start(gnw[t][:], gnw_d.ap()[t])
                    nc.gpsimd.dma_start(gnb[t][:], gnb_d.ap()[t])
                nc.gpsimd.dma_start(bproj_f[:], bproj_d.ap())
                for t in range(2):
                    nc.gpsimd.dma_start(
                        wq16[t][:], wqkvt_d.ap()[t * 128 : t * 128 + 128, :]
                    )
                    nc.gpsimd.dma_start(
                        wp_r[t][:], wpt_d.ap()[t * 128 : t * 128 + 128, :]
                    )

                bno = [
                    stagep.tile([128, 48], F32, tag=f"bno{t}", name=f"bno{t}")
                    for t in range(2)
                ]
                for hf in range(2):
                    hsl = slice(hf * 2048, hf * 2048 + 2048)
                    nc.sync.dma_start(x_sb[0][:, hsl], xb[0:128, hsl])
                    nc.scalar.dma_start(x_sb[1][:, hsl], xb[128:256, hsl])
                    for t in range(2):
                        # keepalive matmul (reads the half; keeps HAM warm)
                        nc.tensor.matmul(
                            wps[0:1, 0:64], ones16[:],
                            x_sb[t][:, hf * 2048 : hf * 2048 + 64],
                            start=True, stop=True,
                        )
                        for j in range(4):
                            ch = 4 * hf + j
                            nc.vector.bn_stats(
                                bno[t][:, ch * 6 : ch * 6 + 6],
                                x_sb[t][:, ch * 512 : ch * 512 + 512],
                            )
                # residual slice (needed only at proj time)
                nc.gpsimd.dma_start(
                    xst[:].rearrange("p (a f) -> p a f", a=8),
                    xst_d.ap().rearrange("(a p) f -> p a f", p=128),
                )

                nc.vector.tensor_copy(onesrow16[:], onesrow[:])
                nc.vector.tensor_copy(ones8[:], onescol[:])
                # ones columns of v^T (softmax denominator accumulators)
                nc.vector.tensor_copy(
                    vt_5d[:, :, :, :, 64:65],
                    ones8[:].to_broadcast([128, 16, 2, 4, 1]),
                )

                # sm = [mean0, E[x^2]0, mean1, E[x^2]1]  [128, 4]
                sm = stagep.tile([128, 4], F32, tag="sm", name="sm")
                for t in range(2):
                    agg = stagep.tile([128, 2], F32, tag=f"agg{t}", name=f"agg{t}")
                    nc.vector.bn_aggr(
                        agg[:], bno[t][:].rearrange("p (c s) -> p c s", c=16)
                    )
                    nc.vector.tensor_copy(sm[:, 2 * t : 2 * t + 1], agg[:, 0:1])
                    msq = stagep.tile([128, 1], F32, tag=f"msq{t}", name=f"msq{t}")
                    nc.vector.tensor_tensor(
                        msq[:], agg[:, 0:1], agg[:, 0:1], op=ALU.mult
                    )
                    nc.vector.tensor_tensor(
                        sm[:, 2 * t + 1 : 2 * t + 2], agg[:, 1:2], msq[:],
                        op=ALU.add,
                    )

                # ---- fused group chain on [16, 4] --------------------
                gp = psgn.tile([16, 4], F32, tag="gp", name="gp")
                nc.tensor.matmul(gp[:], gmap[:], sm[:], start=True, stop=True)
                grs = stagep.tile([16, 4], F32, tag="grs", name="grs")
                nc.vector.tensor_scalar(grs[:], gp[:], 1.0 / GS, None, op0=ALU.mult)
                mu = grs[:].rearrange("p (g s) -> p g s", g=2)[:, :, 0]
                e2 = grs[:].rearrange("p (g s) -> p g s", g=2)[:, :, 1]
                mu2 = stagep.tile([16, 2], F32, tag="mu2", name="mu2")
                nc.vector.tensor_tensor(mu2[:], mu, mu, op=ALU.mult)
                vg = stagep.tile([16, 2], F32, tag="vg", name="vg")
                nc.vector.tensor_tensor(vg[:], e2, mu2[:], op=ALU.subtract)
                ve = stagep.tile([16, 2], F32, tag="ve", name="ve")
                nc.vector.tensor_scalar(ve[:], vg[:], EPS, None, op0=ALU.add)
                mgt = stagep.tile([16, 2], I32, tag="mg", name="mg")
                nc.vector.memset(mgt[:], 0x5F3759DF)
                half = stagep.tile([16, 2], I32, tag="hf", name="hf")
                nc.vector.tensor_scalar(
                    half[:], ve[:].bitcast(I32), 1, None,
                    op0=ALU.logical_shift_right,
                )
                y = stagep.tile([16, 2], F32, tag="qy", name="qy")
                nc.vector.tensor_tensor(
                    y[:].bitcast(I32), mgt[:], half[:], op=ALU.subtract
                )
                for it in range(2):
                    ysq = stagep.tile([16, 2], F32, tag=f"ys{it}", name=f"ys{it}")
                    nc.vector.tensor_tensor(ysq[:], y[:], y[:], op=ALU.mult)
                    vy2 = stagep.tile([16, 2], F32, tag=f"vy{it}", name=f"vy{it}")
                    nc.vector.tensor_tensor(vy2[:], ysq[:], ve[:], op=ALU.mult)
                    hh = stagep.tile([16, 2], F32, tag=f"hh{it}", name=f"hh{it}")
                    nc.vector.tensor_scalar(
                        hh[:], vy2[:], -0.5, 1.5, op0=ALU.mult, op1=ALU.add
                    )
                    if it == 0:
                        yn = stagep.tile([16, 2], F32, tag="yn", name="yn")
                        nc.vector.tensor_tensor(yn[:], y[:], hh[:], op=ALU.mult)
                        y = yn
                    else:
                        # write rstd into grs cols {1, 3}
                        nc.vector.tensor_tensor(e2, y[:], hh[:], op=ALU.mult)

                # ---- per-channel a, b --------------------------------
                a_t = []
                b16 = []
                for t in range(2):
                    bp = psgn.tile([128, 2], F32, tag="bp", name="bp")
                    nc.tensor.matmul(
                        bp[:], gmapt[:], grs[:, 2 * t : 2 * t + 2],
                        start=True, stop=True,
                    )
                    ab = stagep.tile([128, 2], F32, tag=f"ab{t}", name=f"ab{t}")
                    nc.vector.tensor_copy(ab[:], bp[:])
                    av = stagep.tile([128, 1], F32, tag=f"av{t}", name=f"av{t}")
                    nc.vector.tensor_tensor(av[:], ab[:, 1:2], gnw[t][:], op=ALU.mult)
                    tmp = stagep.tile([128, 1], F32, tag=f"tmp{t}", name=f"tmp{t}")
                    nc.vector.tensor_tensor(tmp[:], ab[:, 0:1], av[:], op=ALU.mult)
                    bv = stagep.tile([128, 1], F16, tag=f"bv{t}", name=f"bv{t}")
                    nc.vector.tensor_tensor(bv[:], gnb[t][:], tmp[:], op=ALU.subtract)
                    a_t.append(av)
                    b16.append(bv)

                # ---- fold GN affine into qkv weights -----------------
                # k = (wk*a) @ x + (wk @ b) x 1;  same for q.  The v bias
                # passes through softmax (weights sum to 1) and folds into
                # the projection bias: bproj_eff = bproj + wp @ (wv @ b).
                for ct in range(2):
                    nc.vector.tensor_scalar(
                        wq_a16[ct][:], wq16[ct][:], a_t[ct][:], None, op0=ALU.mult
                    )
                for mt in range(2):
                    for dst, off in ((beta_q[mt], mt * 128),
                                     (beta_k[mt], C + mt * 128)):
                        b_ps = psgn.tile([128, 1], F32, tag="bps", name="bps")
                        for ct in range(2):
                            nc.tensor.matmul(
                                b_ps[:], wq16[ct][:, off : off + 128], b16[ct][:],
                                start=(ct == 0), stop=(ct == 1),
                            )
                        nc.vector.tensor_copy(dst[:], b_ps[:])
                bv_sb = []
                for ot in range(2):
                    b_ps = psgn.tile([128, 1], F32, tag="bps", name="bps")
                    for ct in range(2):
                        nc.tensor.matmul(
                            b_ps[:],
                            wq16[ct][:, 2 * C + ot * 128 : 2 * C + ot * 128 + 128],
                            b16[ct][:],
                            start=(ct == 0), stop=(ct == 1),
                        )
                    bv = stagep.tile([128, 1], F16, tag=f"bvv{ot}", name=f"bvv{ot}")
                    nc.vector.tensor_copy(bv[:], b_ps[:])
                    bv_sb.append(bv)
                row_ps = psgn.tile([1, C], F32, tag="row", name="row")
                for ot in range(2):
                    nc.tensor.matmul(
                        row_ps[:], bv_sb[ot][:], wp_r[ot][:],
                        start=(ot == 0), stop=(ot == 1),
                    )
                bpe = stagep.tile([1, C], F32, tag="bpe", name="bpe")
                nc.vector.tensor_tensor(bpe[:], bproj_f[:], row_ps[:], op=ALU.add)
                nc.vector.tensor_copy(bproj_e16[:], bpe[:])

            # ---- qkv + attention + proj ------------------------------
            exp_idx = [0]
            cast_idx = [0]

            with (
                tc.tile_pool(name="pss", bufs=1, space="PSUM") as pss,
                tc.tile_pool(name="psu", bufs=1, space="PSUM") as psu,
            ):
                def stile():
                    return pss.tile([128, 1024], F32, tag="s", name="s", bufs=3)

                def cast(dst, src, bias=None, force=None):
                    """psum -> sbuf cast (+ per-partition bias) on ACT/DVE."""
                    eng = force if force is not None else (
                        "act" if cast_idx[0] % 2 == 0 else "dve"
                    )
                    cast_idx[0] += 1
                    if eng == "act":
                        if bias is None:
                            nc.scalar.activation(dst, src, AF.Copy)
                        else:
                            nc.scalar.activation(
                                dst, src, AF.Identity, bias=bias
                            )
                    else:
                        if bias is None:
                            nc.vector.tensor_copy(dst, src)
                        else:
                            nc.vector.tensor_scalar(
                                dst, src, bias, None, op0=ALU.add
                            )

                def emit_k_pair(mt, pair, force=None):
                    # k columns [pair*1024, (pair+1)*1024) -> one s tile
                    ps = stile()
                    for half in range(2):
                        csl = slice(pair * 1024 + half * 512,
                                    pair * 1024 + half * 512 + 512)
                        for ct in range(2):
                            nc.tensor.matmul(
                                ps[:, half * 512 : half * 512 + 512],
                                wq_a16[ct][:, C + mt * 128 : C + mt * 128 + 128],
                                x_sb[ct][:, csl],
                                start=(ct == 0),
                                stop=(ct == 1),
                            )
                    cast(k_sb[mt][:, pair * 1024 : pair * 1024 + 1024], ps[:],
                         bias=beta_k[mt][:], force=force)

                def emit_q_both(nch, force=None):
                    # q for both head-pair tiles, one 512-query slice
                    # (queries sit at x columns 0:1024 after the rotation)
                    qsl = slice(nch * 512, nch * 512 + 512)
                    ps = stile()
                    for mt in range(2):
                        for ct in range(2):
                            nc.tensor.matmul(
                                ps[:, mt * 512 : mt * 512 + 512],
                                wq_a16[ct][:, mt * 128 : mt * 128 + 128],
                                x_sb[ct][:, qsl],
                                start=(ct == 0),
                                stop=(ct == 1),
                            )
                    for mt in range(2):
                        cast(q_sb[mt][:, qsl], ps[:, mt * 512 : mt * 512 + 512],
                             bias=beta_q[mt][:], force=force)

                def emit_v_quad(cp, force=None):
                    # v^T for key chunks 4cp..4cp+3 -> one s tile
                    ps = stile()
                    for ii in range(4):
                        mch = 4 * cp + ii
                        for ct in range(2):
                            nc.tensor.matmul(
                                ps[:, ii * 256 : ii * 256 + 256],
                                x_sb[ct][:, mch * 128 : mch * 128 + 128],
                                wq_a16[ct][:, 2 * C : 3 * C],
                                start=(ct == 0),
                                stop=(ct == 1),
                            )
                    for half in range(2):
                        cast(
                            vt_5d[:, 2 * cp + half, :, :, 0:64],
                            ps[:, half * 512 : half * 512 + 512].rearrange(
                                "p (i h m) -> p i h m", i=2, h=4
                            ),
                            force=force,
                        )

                def emit_exp(s_ps, p8p, i):
                    in_view = s_ps[:].rearrange("p (h n) -> p h n", h=2)
                    out_view = p8p[:].rearrange(
                        "p (h i n) -> p h i n", h=2, i=2
                    )[:, :, i, :]
                    if exp_on_act(exp_idx[0]):
                        nc.scalar.activation(
                            out_view, in_view, AF.Exp, bias=biasm3[:], scale=SCALE
                        )
                    else:
                        nc.vector.tensor_scalar(
                            out_view.bitcast(U8), in_view, A8, B8,
                            op0=ALU.mult, op1=ALU.add,
                        )
                    exp_idx[0] += 1

                def attention_block(nch, hp, extra):
                    """extra: dict {c: callback} interleaved into the c loop.
                    AV matmuls trail the scores by 2 c-groups so the PE
                    (in-order queue) never stalls on exp completion."""
                    qsl = slice(nch * 512, nch * 512 + 512)
                    u2 = [
                        psu.tile([65, 512], F32, tag=f"u{j}", name=f"u{j}", bufs=1)
                        for j in range(2)
                    ]

                    def emit_av(c, p8p):
                        for j in range(2):
                            nc.tensor.matmul(
                                u2[j][:],
                                vt_5d[:, c, :, 2 * hp + j, 0:65],
                                p8p[:].rearrange(
                                    "p (h i n) -> p h i n", h=2, i=2
                                )[:, j, :, :],
                                start=(c == 0),
                                stop=(c == 15),
                                perf_mode=MPM.DoubleRow,
                            )

                    pend = []
                    for c in range(16):
                        if c in extra:
                            extra[c]()
                        p8p = rotp.tile(
                            [128, 2048], F8, tag="p8", name="p8", bufs=4
                        )
                        for i in range(2):
                            mch = 2 * c + i
                            msl = slice(mch * 128, mch * 128 + 128)
                            s_ps = stile()
                            nc.tensor.matmul(
                                s_ps[:, 0:512],
                                k_sb[hp][0:64, msl],
                                q_sb[hp][0:64, qsl],
                                start=True, stop=True,
                            )
                            nc.tensor.matmul(
                                s_ps[:, 512:1024],
                                k_sb[hp][64:128, msl],
                                q_sb[hp][64:128, qsl],
                                start=True, stop=True,
                            )
                            emit_exp(s_ps, p8p, i)
                        pend.append((c, p8p))
                        if DEBUG_DUMPS and nch == 0 and hp == 0 and c == 0:
                            nc.sync.dma_start(dp8_d.ap(), p8p[:].bitcast(U8))
                        if len(pend) > 2:
                            emit_av(*pend.pop(0))
                    for item in pend:
                        emit_av(*item)

                    # normalize: u_sb[hp][64j:64j+64, qsl] = u2[0:64] / u2[64]
                    for j in range(2):
                        lh = rotp.tile([1, 512], F32, tag="lh", name="lh")
                        nc.vector.tensor_copy(lh[:], u2[j][64:65, :])
                        rh = rotp.tile([1, 512], F32, tag="rh", name="rh")
                        nc.vector.reciprocal_approx_fast(rh[:], lh[:])
                        rb = rotp.tile([64, 512], F32, tag="rb", name="rb")
                        nc.gpsimd.partition_broadcast(rb[:], rh[:])
                        nc.vector.tensor_tensor(
                            u_sb[hp][64 * j : 64 * j + 64, qsl],
                            u2[j][0:64, :],
                            rb[:],
                            op=ALU.mult,
                        )
                        if DEBUG_DUMPS and nch == 0 and hp == 0:
                            u2c = rotp.tile([65, 512], F32, tag="u2c", name="u2c")
                            nc.vector.tensor_copy(u2c[:], u2[j][:])
                            nc.sync.dma_start(du2_d.ap()[j], u2c[:])
                            nc.sync.dma_start(drb_d.ap()[j], rb[:])

                def emit_proj(nc8):
                    sl = slice(nc8 * 128, nc8 * 128 + 128)
                    ps = stile()
                    y_ps = ps[:, 0:256]
                    for ct in range(2):
                        nc.tensor.matmul(
                            y_ps,
                            u_sb[ct][:, sl],
                            wp_r[ct][:],
                            start=(ct == 0),
                            stop=False,
                        )
                    nc.tensor.matmul(
                        y_ps, onesrow16[:], bproj_e16[:], start=False, stop=True
                    )
                    out_t = rotp.tile([128, C], F32, tag="out", name="out")
                    nc.vector.tensor_tensor(
                        out_t[:], y_ps, xst[:, nc8 * C : nc8 * C + C], op=ALU.add
                    )
                    nc.sync.dma_start(yt[sl, :], out_t[:])

                # strictly-needed prologue: k tile 0, q (nch 0); casts on
                # ACT (DVE is still finishing stats at this point)
                for pair in range(4):
                    emit_k_pair(0, pair, force="act")
                emit_q_both(0, force="act")

                # block (nch0, hp0): interleave v quads, q-nch1, k tile 1
                extra0 = {cp: (lambda cp=cp: emit_v_quad(cp)) for cp in range(8)}
                extra0[8] = lambda: emit_q_both(1)
                extra0[9] = lambda: emit_k_pair(1, 0)
                extra0[11] = lambda: emit_k_pair(1, 1)
                extra0[13] = lambda: emit_k_pair(1, 2)
                extra0[15] = lambda: emit_k_pair(1, 3)
                attention_block(0, 0, extra0)
                attention_block(0, 1, {})

                # block (nch1, hp0): interleave proj of nch0 (away from the
                # block start: proj waits on the previous normalize)
                extra2 = {2 + 2 * n: (lambda n=n: emit_proj(n)) for n in range(4)}
                attention_block(1, 0, extra2)
                attention_block(1, 1, {})
                for nc8 in range(4, 8):
                    emit_proj(nc8)

                if DEBUG_DUMPS:
                    nc.sync.dma_start(dvt_d.ap(), vt[:].bitcast(U8))
                    for t in range(2):
                        nc.sync.dma_start(dk_d.ap()[t], k_sb[t][:])
                        nc.sync.dma_start(dq_d.ap()[t], q_sb[t][:])
                        nc.sync.dma_start(du_d.ap()[t], u_sb[t][:])

    nc.compile()
    return nc


def _in_maps(inputs):
    x = np.asarray(inputs["x"], dtype=np.float32)
    gn_scale = np.asarray(inputs["gn_scale"], dtype=np.float32)
    gn_bias = np.asarray(inputs["gn_bias"], dtype=np.float32)
    w_qkv = np.asarray(inputs["w_qkv"], dtype=np.float32)
    w_proj = np.asarray(inputs["w_proj"], dtype=np.float32)
    b_proj = np.asarray(inputs["b_proj"], dtype=np.float32)

    B = x.shape[0]
    xf = x.reshape(B, C, N).astype(np.float16)
    wqkvt = np.ascontiguousarray(w_qkv.T.astype(np.float16))
    wpt = np.ascontiguousarray(w_proj.T.astype(np.float16))
    gnw = np.ascontiguousarray(gn_scale.reshape(2, 128, 1))
    gnb = np.ascontiguousarray(gn_bias.reshape(2, 128, 1))
    bproj = np.ascontiguousarray(b_proj.reshape(1, C))
    gmap = np.zeros((128, 16), dtype=np.float32)
    gmap[np.arange(128), np.arange(128) // GS] = 1.0
    gmapt = np.ascontiguousarray(gmap.T)
    onescol = np.ones((128, 1), dtype=np.float32)
    onesrow = np.ones((1, 128), dtype=np.float32)

    maps = []
    for core in range(8):
        b, s = core // 4, core % 4
        # rotate so this core's query slice sits at columns 0:1024
        # (key order is attention-invariant; k/v use the same order)
        xr = np.ascontiguousarray(np.roll(xf[b], -s * NS, axis=1))
        maps.append(
            {
                "xb": xr,
                "xst": np.ascontiguousarray(xr[:, 0:NS].T),
                "wqkvt": wqkvt,
                "wpt": wpt,
                "gnw": gnw,
                "gnb": gnb,
                "bproj": bproj,
                "gmap": gmap,
                "gmapt": gmapt,
                "onescol": onescol,
                "onesrow": onesrow,
            }
        )
    return maps


def _run(inputs, trace=False):
    if "nc" not in _cached:
        _cached["nc"] = _build()
    nc = _cached["nc"]
    maps = _in_maps(inputs)
    res = None
    for attempt in range(4):
        try:
            res = bass_utils.run_bass_kernel_spmd(
                nc, maps, core_ids=list(range(8)), trace=trace
            )
            break
        except Exception:
            if attempt == 3:
                raise
            time.sleep(10.0 * (attempt + 1))
    outs = np.stack([res.results[c]["yt"] for c in range(8)])  # [8, NS, C]
    y = outs.reshape(2, 4 * NS, C).transpose(0, 2, 1).reshape(2, C, 64, 64)
    return np.ascontiguousarray(y.astype(np.float32)), res


def kernel(**inputs):
    y, _ = _run(inputs, trace=False)
    return y


# revision 30
# speedup vs baseline: 1.3948x; 1.1272x over previous
"""AttentionBlock (GroupNorm + 4-head self-attention + proj + residual) on 8
Trainium2 NeuronCores.

Sharding: core i handles batch b = i // 4 and query slice s = i % 4 (1024 of
4096 query positions).  Each core computes full k/v for its batch (replicated
within the 4 cores of a batch), attention for all 4 heads over its query
slice, and the output projection + residual for its slice.  Outputs are
disjoint [1024, 256] (query-major) slices; the host concatenates and
transposes back to [2, 256, 64, 64].

Key design points:
  - Host sends x / weights / residual in f16 and ROTATES each core's x so
    its query slice sits at columns 0:1024 (key order is attention-
    invariant): 2.7 MB of demand-critical DMA instead of 6.2 MB.
  - The GroupNorm affine folds into the qkv weights: k = (wk*a) @ x + bk x 1
    (bk via tiny matmuls of b against the weights); the v bias passes
    through softmax (weights sum to 1) and folds into the projection bias.
    So qkv matmuls consume raw f16 x chunks as they land.
  - softmax exp splits across ScalarE (exact exp -> fp8e4) and VectorE
    (Schraudolph exp: one tensor_scalar f32->u8 whose bits ARE the fp8e4
    weight; HW rounds-to-nearest and saturates at 0 = free clamp).
  - attn @ v runs as fp8 DoubleRow matmuls (256-key contraction), halving
    the dominant PE stream time; a ones column in v^T accumulates the
    softmax denominators.
  - The PE queue is in-order, so AV matmuls trail their scores by 2 chunk
    groups, v/k1/q1/proj work is interleaved into the attention stream,
    and warmup + DMA-gated keepalive matmuls hold the HAM clock-gate open.
"""

import sys
import time

if "/opt/trn_rl_repo" not in sys.path:
    sys.path.insert(0, "/opt/trn_rl_repo")

import numpy as np

import concourse.bacc as bacc
import concourse.tile as tile
import concourse.mybir as mybir
from concourse import bass_utils

F32 = mybir.dt.float32
F16 = mybir.dt.float16
F8 = mybir.dt.float8e4
I32 = mybir.dt.int32
U8 = mybir.dt.uint8
AF = mybir.ActivationFunctionType
ALU = mybir.AluOpType
MPM = mybir.MatmulPerfMode

C = 256  # channels
N = 4096  # h*w
NS = 1024  # query slice per core
H = 4  # heads
HD = 64  # head dim
G = 32  # groups
GS = 8  # channels per group
EPS = 1e-5
SCALE = HD**-0.5  # 0.125

# Schraudolph exp on raw scores s: fp8e4 bits = rne(clip(A8*s + B8, 0, 255))
# approximates exp(s*SCALE - 3).  (8/ln2)*SCALE = 1.4427066;
# bias 56 = 7*8 (fp8e4 exponent bias), minus the 3-shift in exponent units.
A8 = 1.4427066
B8 = 56.0 - 3.0 * (8.0 / 0.6931471805599453)

_cached = {}
DEBUG_DUMPS = False


def _build():
    nc = bacc.Bacc("TRN2", target_bir_lowering=False, debug=False, num_devices=8)

    xb_d = nc.dram_tensor("xb", [C, N], F16, kind="ExternalInput")
    xst_d = nc.dram_tensor("xst", [NS, C], F16, kind="ExternalInput")
    wqkvt_d = nc.dram_tensor("wqkvt", [C, 3 * C], F16, kind="ExternalInput")
    wpt_d = nc.dram_tensor("wpt", [C, C], F16, kind="ExternalInput")
    gnw_d = nc.dram_tensor("gnw", [2, 128, 1], F32, kind="ExternalInput")
    gnb_d = nc.dram_tensor("gnb", [2, 128, 1], F32, kind="ExternalInput")
    bproj_d = nc.dram_tensor("bproj", [1, C], F32, kind="ExternalInput")
    gmap_d = nc.dram_tensor("gmap", [128, 16], F32, kind="ExternalInput")
    gmapt_d = nc.dram_tensor("gmapt", [16, 128], F32, kind="ExternalInput")
    onescol_d = nc.dram_tensor("onescol", [128, 1], F32, kind="ExternalInput")
    onesrow_d = nc.dram_tensor("onesrow", [1, 128], F32, kind="ExternalInput")
    yt_d = nc.dram_tensor("yt", [NS, C], F32, kind="ExternalOutput")
    if DEBUG_DUMPS:
        dvt_d = nc.dram_tensor("d_vt", [128, 16 * 2 * 4 * 68], U8, kind="ExternalOutput")
        dk_d = nc.dram_tensor("d_k", [2, 128, N], F16, kind="ExternalOutput")
        dq_d = nc.dram_tensor("d_q", [2, 128, NS], F16, kind="ExternalOutput")
        du_d = nc.dram_tensor("d_u", [2, 128, NS], F16, kind="ExternalOutput")
        du2_d = nc.dram_tensor("d_u2", [2, 65, 512], F32, kind="ExternalOutput")
        drb_d = nc.dram_tensor("d_rb", [2, 64, 512], F32, kind="ExternalOutput")
        dp8_d = nc.dram_tensor("d_p8", [128, 2048], U8, kind="ExternalOutput")

    xb = xb_d.ap()
    yt = yt_d.ap()

    # exp engine assignment: True -> ScalarE (exact), False -> DVE
    # Schraudolph.  Finely interleaved (the pipeline is only ~2 c-groups
    # deep, so coarse runs starve one engine): i=0 -> ACT, i=1 -> DVE,
    # plus both on ACT every 8th group (~9/16 on ACT).
    def exp_on_act(c, i):
        return i == 0 or c % 8 == 0

    with tile.TileContext(nc) as tc:
        with (
            tc.tile_pool(name="const", bufs=1) as constp,
            tc.tile_pool(name="main", bufs=1) as mainp,
            tc.tile_pool(name="rot", bufs=3) as rotp,
        ):
            # ---- persistent tiles ------------------------------------
            gmap = constp.tile([128, 16], F32, tag="gmap", name="gmap")
            gmapt = constp.tile([16, 128], F32, tag="gmapt", name="gmapt")
            onescol = constp.tile([128, 1], F32, tag="onescol", name="onescol")
            ones16 = constp.tile([128, 1], F16, tag="ones16", name="ones16")
            onesrow = constp.tile([1, 128], F32, tag="onesrow", name="onesrow")
            onesrow16 = constp.tile([1, 128], F16, tag="onesrow16", name="onesrow16")
            ones8 = constp.tile([128, 1], F8, tag="ones8", name="ones8")
            bproj_f = constp.tile([1, C], F32, tag="bproj_f", name="bproj_f")
            biasm3 = constp.tile([128, 1], F32, tag="biasm3", name="biasm3")
            gnw = [constp.tile([128, 1], F32, tag=f"gnw{t}", name=f"gnw{t}") for t in range(2)]
            gnb = [constp.tile([128, 1], F32, tag=f"gnb{t}", name=f"gnb{t}") for t in range(2)]

            # x in f16, per-core rotated (queries at cols 0:1024)
            x_sb = [mainp.tile([128, N], F16, tag=f"x{t}", name=f"x{t}") for t in range(2)]
            k_sb = [mainp.tile([128, N], F16, tag=f"k{t}", name=f"k{t}") for t in range(2)]
            q_sb = [mainp.tile([128, NS], F16, tag=f"q{t}", name=f"q{t}") for t in range(2)]
            # v^T fp8, DoubleRow interleaved: [128, c(16), i(2), h(4), m(68)]
            # (m: 64 head-dim cols + ones col at 64; pitch 68 for 16B align)
            vt = mainp.tile([128, 16 * 2 * 4 * 68], F8, tag="vt", name="vt")
            vt_5d = vt[:].rearrange("p (c i h m) -> p c i h m", c=16, i=2, h=4)
            u_sb = [mainp.tile([128, NS], F16, tag=f"u{t}", name=f"u{t}") for t in range(2)]
            xst = mainp.tile([128, 8 * C], F16, tag="xst", name="xst")
            wq16 = [
                mainp.tile([128, 3 * C], F16, tag=f"wq{t}", name=f"wq{t}")
                for t in range(2)
            ]
            wp_r = [
                mainp.tile([128, C], F16, tag=f"wp{t}", name=f"wp{t}")
                for t in range(2)
            ]
            wq_a16 = [
                constp.tile([128, 3 * C], F16, tag=f"wqa{t}", name=f"wqa{t}")
                for t in range(2)
            ]
            beta_k = [constp.tile([128, 1], F32, tag=f"bk{t}", name=f"bk{t}") for t in range(2)]
            beta_q = [constp.tile([128, 1], F32, tag=f"bq{t}", name=f"bq{t}") for t in range(2)]
            bproj_e16 = constp.tile([1, C], F16, tag="bpe16", name="bpe16")

            with (
                tc.tile_pool(name="stage", bufs=1) as stagep,
                tc.tile_pool(name="wps", bufs=1, space="PSUM") as wpsp,
                tc.tile_pool(name="psgn", bufs=1, space="PSUM") as psgn,
            ):
                # ---- PE warmup (HAM clock-gate) ----------------------
                junk16 = stagep.tile([128, 512], F16, tag="junk", name="junk")
                nc.vector.memset(junk16[:], 0.5)
                wps = wpsp.tile([128, 512], F32, tag="w", name="w")
                for r in range(16):
                    nc.tensor.matmul(
                        wps[:], junk16[:, 0:128], junk16[:], start=True, stop=True
                    )

                # prefetch the exp activation table set
                dummy = stagep.tile([1, 1], F32, tag="dummy", name="dummy")
                nc.vector.memset(dummy[:], 1.0)
                nc.scalar.activation(dummy[:], dummy[:], AF.Exp)
                nc.vector.memset(biasm3[:], -3.0)
                nc.vector.memset(ones16[:], 1.0)

                # ---- consts + weights on gpsimd SWDGE; x halves on the
                # two hwdge queues (fabric is shared; fewest bytes wins) -
                nc.gpsimd.dma_start(gmap[:], gmap_d.ap())
                nc.gpsimd.dma_start(gmapt[:], gmapt_d.ap())
                nc.gpsimd.dma_start(onescol[:], onescol_d.ap())
                nc.gpsimd.dma_start(onesrow[:], onesrow_d.ap())
                for t in range(2):
                    nc.gpsimd.dma_start(gnw[t][:], gnw_d.ap()[t])
                    nc.gpsimd.dma_start(gnb[t][:], gnb_d.ap()[t])
                nc.gpsimd.dma_start(bproj_f[:], bproj_d.ap())
                for t in range(2):
                    nc.gpsimd.dma_start(
                        wq16[t][:], wqkvt_d.ap()[t * 128 : t * 128 + 128, :]
                    )
                    nc.gpsimd.dma_start(
                        wp_r[t][:], wpt_d.ap()[t * 128 : t * 128 + 128, :]
                    )

                bno = stagep.tile([128, 48], F32, tag="bno0", name="bno0")
                sxs = stagep.tile([128, 4], F32, tag="sxs", name="sxs")
                scr = stagep.tile([128, 2048], F16, tag="scr", name="scr")
                for hf in range(2):
                    hsl = slice(hf * 2048, hf * 2048 + 2048)
                    nc.sync.dma_start(x_sb[0][:, hsl], xb[0:128, hsl])
                    nc.scalar.dma_start(x_sb[1][:, hsl], xb[128:256, hsl])
                    for t in range(2):
                        # keepalive matmul (reads the half; keeps HAM warm)
                        nc.tensor.matmul(
                            wps[0:1, 0:64], ones16[:],
                            x_sb[t][:, hf * 2048 : hf * 2048 + 64],
                            start=True, stop=True,
                        )
                    # tile0 stats on DVE
                    for j in range(4):
                        ch = 4 * hf + j
                        nc.vector.bn_stats(
                            bno[:, ch * 6 : ch * 6 + 6],
                            x_sb[0][:, ch * 512 : ch * 512 + 512],
                        )
                    # tile1 stats on ScalarE (accumulators)
                    nc.scalar.activation(
                        scr[:], x_sb[1][:, hsl], AF.Identity,
                        accum_out=sxs[:, hf : hf + 1],
                    )
                    nc.scalar.activation(
                        scr[:], x_sb[1][:, hsl], AF.Square,
                        accum_out=sxs[:, 2 + hf : 3 + hf],
                    )
                # residual slice (needed only at proj time)
                nc.gpsimd.dma_start(
                    xst[:].rearrange("p (a f) -> p a f", a=8),
                    xst_d.ap().rearrange("(a p) f -> p a f", p=128),
                )

                nc.vector.tensor_copy(onesrow16[:], onesrow[:])
                nc.vector.tensor_copy(ones8[:], onescol[:])
                # ones columns of v^T (softmax denominator accumulators)
                nc.vector.tensor_copy(
                    vt_5d[:, :, :, :, 64:65],
                    ones8[:].to_broadcast([128, 16, 2, 4, 1]),
                )

                # sm = [mean0, E[x^2]0, mean1, E[x^2]1]  [128, 4]
                sm = stagep.tile([128, 4], F32, tag="sm", name="sm")
                agg = stagep.tile([128, 2], F32, tag="agg0", name="agg0")
                nc.vector.bn_aggr(agg[:], bno[:].rearrange("p (c s) -> p c s", c=16))
                nc.vector.tensor_copy(sm[:, 0:1], agg[:, 0:1])
                msq = stagep.tile([128, 1], F32, tag="msq0", name="msq0")
                nc.vector.tensor_tensor(msq[:], agg[:, 0:1], agg[:, 0:1], op=ALU.mult)
                nc.vector.tensor_tensor(sm[:, 1:2], agg[:, 1:2], msq[:], op=ALU.add)
                ssum = stagep.tile([128, 2], F32, tag="ssum", name="ssum")
                nc.vector.tensor_reduce(
                    ssum[:], sxs[:].rearrange("p (a c) -> p a c", a=2),
                    axis=mybir.AxisListType.X, op=ALU.add,
                )
                nc.vector.tensor_scalar(
                    sm[:, 2:4], ssum[:], 1.0 / N, None, op0=ALU.mult
                )

                # ---- fused group chain on [16, 4] --------------------
                gp = psgn.tile([16, 4], F32, tag="gp", name="gp")
                nc.tensor.matmul(gp[:], gmap[:], sm[:], start=True, stop=True)
                grs = stagep.tile([16, 4], F32, tag="grs", name="grs")
                nc.vector.tensor_scalar(grs[:], gp[:], 1.0 / GS, None, op0=ALU.mult)
                mu = grs[:].rearrange("p (g s) -> p g s", g=2)[:, :, 0]
                e2 = grs[:].rearrange("p (g s) -> p g s", g=2)[:, :, 1]
                mu2 = stagep.tile([16, 2], F32, tag="mu2", name="mu2")
                nc.vector.tensor_tensor(mu2[:], mu, mu, op=ALU.mult)
                vg = stagep.tile([16, 2], F32, tag="vg", name="vg")
                nc.vector.tensor_tensor(vg[:], e2, mu2[:], op=ALU.subtract)
                ve = stagep.tile([16, 2], F32, tag="ve", name="ve")
                nc.vector.tensor_scalar(ve[:], vg[:], EPS, None, op0=ALU.add)
                mgt = stagep.tile([16, 2], I32, tag="mg", name="mg")
                nc.vector.memset(mgt[:], 0x5F3759DF)
                half = stagep.tile([16, 2], I32, tag="hf", name="hf")
                nc.vector.tensor_scalar(
                    half[:], ve[:].bitcast(I32), 1, None,
                    op0=ALU.logical_shift_right,
                )
                y = stagep.tile([16, 2], F32, tag="qy", name="qy")
                nc.vector.tensor_tensor(
                    y[:].bitcast(I32), mgt[:], half[:], op=ALU.subtract
                )
                for it in range(2):
                    ysq = stagep.tile([16, 2], F32, tag=f"ys{it}", name=f"ys{it}")
                    nc.vector.tensor_tensor(ysq[:], y[:], y[:], op=ALU.mult)
                    vy2 = stagep.tile([16, 2], F32, tag=f"vy{it}", name=f"vy{it}")
                    nc.vector.tensor_tensor(vy2[:], ysq[:], ve[:], op=ALU.mult)
                    hh = stagep.tile([16, 2], F32, tag=f"hh{it}", name=f"hh{it}")
                    nc.vector.tensor_scalar(
                        hh[:], vy2[:], -0.5, 1.5, op0=ALU.mult, op1=ALU.add
                    )
                    if it == 0:
                        yn = stagep.tile([16, 2], F32, tag="yn", name="yn")
                        nc.vector.tensor_tensor(yn[:], y[:], hh[:], op=ALU.mult)
                        y = yn
                    else:
                        # write rstd into grs cols {1, 3}
                        nc.vector.tensor_tensor(e2, y[:], hh[:], op=ALU.mult)

                # ---- per-channel a, b --------------------------------
                a_t = []
                b16 = []
                for t in range(2):
                    bp = psgn.tile([128, 2], F32, tag="bp", name="bp")
                    nc.tensor.matmul(
                        bp[:], gmapt[:], grs[:, 2 * t : 2 * t + 2],
                        start=True, stop=True,
                    )
                    ab = stagep.tile([128, 2], F32, tag=f"ab{t}", name=f"ab{t}")
                    nc.vector.tensor_copy(ab[:], bp[:])
                    av = stagep.tile([128, 1], F32, tag=f"av{t}", name=f"av{t}")
                    nc.vector.tensor_tensor(av[:], ab[:, 1:2], gnw[t][:], op=ALU.mult)
                    tmp = stagep.tile([128, 1], F32, tag=f"tmp{t}", name=f"tmp{t}")
                    nc.vector.tensor_tensor(tmp[:], ab[:, 0:1], av[:], op=ALU.mult)
                    bv = stagep.tile([128, 1], F16, tag=f"bv{t}", name=f"bv{t}")
                    nc.vector.tensor_tensor(bv[:], gnb[t][:], tmp[:], op=ALU.subtract)
                    a_t.append(av)
                    b16.append(bv)

                # ---- fold GN affine into qkv weights -----------------
                # k = (wk*a) @ x + (wk @ b) x 1;  same for q.  The v bias
                # passes through softmax (weights sum to 1) and folds into
                # the projection bias: bproj_eff = bproj + wp @ (wv @ b).
                for ct in range(2):
                    nc.vector.tensor_scalar(
                        wq_a16[ct][:], wq16[ct][:], a_t[ct][:], None, op0=ALU.mult
                    )
                for mt in range(2):
                    for dst, off in ((beta_q[mt], mt * 128),
                                     (beta_k[mt], C + mt * 128)):
                        b_ps = psgn.tile([128, 1], F32, tag="bps", name="bps")
                        for ct in range(2):
                            nc.tensor.matmul(
                                b_ps[:], wq16[ct][:, off : off + 128], b16[ct][:],
                                start=(ct == 0), stop=(ct == 1),
                            )
                        nc.vector.tensor_copy(dst[:], b_ps[:])
                bv_sb = []
                for ot in range(2):
                    b_ps = psgn.tile([128, 1], F32, tag="bps", name="bps")
                    for ct in range(2):
                        nc.tensor.matmul(
                            b_ps[:],
                            wq16[ct][:, 2 * C + ot * 128 : 2 * C + ot * 128 + 128],
                            b16[ct][:],
                            start=(ct == 0), stop=(ct == 1),
                        )
                    bv = stagep.tile([128, 1], F16, tag=f"bvv{ot}", name=f"bvv{ot}")
                    nc.vector.tensor_copy(bv[:], b_ps[:])
                    bv_sb.append(bv)
                row_ps = psgn.tile([1, C], F32, tag="row", name="row")
                for ot in range(2):
                    nc.tensor.matmul(
                        row_ps[:], bv_sb[ot][:], wp_r[ot][:],
                        start=(ot == 0), stop=(ot == 1),
                    )
                bpe = stagep.tile([1, C], F32, tag="bpe", name="bpe")
                nc.vector.tensor_tensor(bpe[:], bproj_f[:], row_ps[:], op=ALU.add)
                nc.vector.tensor_copy(bproj_e16[:], bpe[:])

            # ---- qkv + attention + proj ------------------------------
            exp_idx = [0]
            cast_idx = [0]

            with (
                tc.tile_pool(name="pss", bufs=1, space="PSUM") as pss,
                tc.tile_pool(name="psu", bufs=1, space="PSUM") as psu,
            ):
                def stile():
                    return pss.tile([128, 1024], F32, tag="s", name="s", bufs=3)

                def cast(dst, src, bias=None, force=None):
                    """psum -> sbuf cast (+ per-partition bias) on ACT/DVE."""
                    eng = force if force is not None else (
                        "act" if cast_idx[0] % 2 == 0 else "dve"
                    )
                    cast_idx[0] += 1
                    if eng == "act":
                        if bias is None:
                            nc.scalar.activation(dst, src, AF.Copy)
                        else:
                            nc.scalar.activation(
                                dst, src, AF.Identity, bias=bias
                            )
                    else:
                        if bias is None:
                            nc.vector.tensor_copy(dst, src)
                        else:
                            nc.vector.tensor_scalar(
                                dst, src, bias, None, op0=ALU.add
                            )

                def emit_k_pair(mt, pair, force=None):
                    # k columns [pair*1024, (pair+1)*1024) -> one s tile
                    ps = stile()
                    for half in range(2):
                        csl = slice(pair * 1024 + half * 512,
                                    pair * 1024 + half * 512 + 512)
                        for ct in range(2):
                            nc.tensor.matmul(
                                ps[:, half * 512 : half * 512 + 512],
                                wq_a16[ct][:, C + mt * 128 : C + mt * 128 + 128],
                                x_sb[ct][:, csl],
                                start=(ct == 0),
                                stop=(ct == 1),
                            )
                    cast(k_sb[mt][:, pair * 1024 : pair * 1024 + 1024], ps[:],
                         bias=beta_k[mt][:], force=force)

                def emit_q_both(nch, force=None):
                    # q for both head-pair tiles, one 512-query slice
                    # (queries sit at x columns 0:1024 after the rotation)
                    qsl = slice(nch * 512, nch * 512 + 512)
                    ps = stile()
                    for mt in range(2):
                        for ct in range(2):
                            nc.tensor.matmul(
                                ps[:, mt * 512 : mt * 512 + 512],
                                wq_a16[ct][:, mt * 128 : mt * 128 + 128],
                                x_sb[ct][:, qsl],
                                start=(ct == 0),
                                stop=(ct == 1),
                            )
                    for mt in range(2):
                        cast(q_sb[mt][:, qsl], ps[:, mt * 512 : mt * 512 + 512],
                             bias=beta_q[mt][:], force=force)

                def emit_v_quad(cp, force=None):
                    # v^T for key chunks 4cp..4cp+3 -> one s tile
                    ps = stile()
                    for ii in range(4):
                        mch = 4 * cp + ii
                        for ct in range(2):
                            nc.tensor.matmul(
                                ps[:, ii * 256 : ii * 256 + 256],
                                x_sb[ct][:, mch * 128 : mch * 128 + 128],
                                wq_a16[ct][:, 2 * C : 3 * C],
                                start=(ct == 0),
                                stop=(ct == 1),
                            )
                    for half in range(2):
                        cast(
                            vt_5d[:, 2 * cp + half, :, :, 0:64],
                            ps[:, half * 512 : half * 512 + 512].rearrange(
                                "p (i h m) -> p i h m", i=2, h=4
                            ),
                            force=force,
                        )

                def emit_exp(s_ps, p8p, c, i):
                    in_view = s_ps[:].rearrange("p (h n) -> p h n", h=2)
                    out_view = p8p[:].rearrange(
                        "p (h i n) -> p h i n", h=2, i=2
                    )[:, :, i, :]
                    if exp_on_act(c, i):
                        nc.scalar.activation(
                            out_view, in_view, AF.Exp, bias=biasm3[:], scale=SCALE
                        )
                    else:
                        nc.vector.tensor_scalar(
                            out_view.bitcast(U8), in_view, A8, B8,
                            op0=ALU.mult, op1=ALU.add,
                        )
                    exp_idx[0] += 1

                def attention_block(nch, hp, extra):
                    """extra: dict {c: callback} interleaved into the c loop.
                    AV matmuls trail the scores by 2 c-groups so the PE
                    (in-order queue) never stalls on exp completion."""
                    qsl = slice(nch * 512, nch * 512 + 512)
                    u2 = [
                        psu.tile([65, 512], F32, tag=f"u{j}", name=f"u{j}", bufs=1)
                        for j in range(2)
                    ]

                    def emit_av(c, p8p):
                        for j in range(2):
                            nc.tensor.matmul(
                                u2[j][:],
                                vt_5d[:, c, :, 2 * hp + j, 0:65],
                                p8p[:].rearrange(
                                    "p (h i n) -> p h i n", h=2, i=2
                                )[:, j, :, :],
                                start=(c == 0),
                                stop=(c == 15),
                                perf_mode=MPM.DoubleRow,
                            )

                    pend = []
                    for c in range(16):
                        if c in extra:
                            extra[c]()
                        p8p = rotp.tile(
                            [128, 2048], F8, tag="p8", name="p8", bufs=4
                        )
                        for i in range(2):
                            mch = 2 * c + i
                            msl = slice(mch * 128, mch * 128 + 128)
                            s_ps = stile()
                            nc.tensor.matmul(
                                s_ps[:, 0:512],
                                k_sb[hp][0:64, msl],
                                q_sb[hp][0:64, qsl],
                                start=True, stop=True,
                            )
                            nc.tensor.matmul(
                                s_ps[:, 512:1024],
                                k_sb[hp][64:128, msl],
                                q_sb[hp][64:128, qsl],
                                start=True, stop=True,
                            )
                            emit_exp(s_ps, p8p, c, i)
                        pend.append((c, p8p))
                        if DEBUG_DUMPS and nch == 0 and hp == 0 and c == 0:
                            nc.sync.dma_start(dp8_d.ap(), p8p[:].bitcast(U8))
                        if len(pend) > 2:
                            emit_av(*pend.pop(0))
                    for item in pend:
                        emit_av(*item)

                    # normalize: u_sb[hp][64j:64j+64, qsl] = u2[0:64] / u2[64]
                    for j in range(2):
                        lh = rotp.tile([1, 512], F32, tag="lh", name="lh")
                        nc.vector.tensor_copy(lh[:], u2[j][64:65, :])
                        rh = rotp.tile([1, 512], F32, tag="rh", name="rh")
                        nc.vector.reciprocal_approx_fast(rh[:], lh[:])
                        rb = rotp.tile([64, 512], F32, tag="rb", name="rb")
                        nc.gpsimd.partition_broadcast(rb[:], rh[:])
                        nc.vector.tensor_tensor(
                            u_sb[hp][64 * j : 64 * j + 64, qsl],
                            u2[j][0:64, :],
                            rb[:],
                            op=ALU.mult,
                        )
                        if DEBUG_DUMPS and nch == 0 and hp == 0:
                            u2c = rotp.tile([65, 512], F32, tag="u2c", name="u2c")
                            nc.vector.tensor_copy(u2c[:], u2[j][:])
                            nc.sync.dma_start(du2_d.ap()[j], u2c[:])
                            nc.sync.dma_start(drb_d.ap()[j], rb[:])

                def emit_proj(nc8):
                    sl = slice(nc8 * 128, nc8 * 128 + 128)
                    ps = stile()
                    y_ps = ps[:, 0:256]
                    for ct in range(2):
                        nc.tensor.matmul(
                            y_ps,
                            u_sb[ct][:, sl],
                            wp_r[ct][:],
                            start=(ct == 0),
                            stop=False,
                        )
                    nc.tensor.matmul(
                        y_ps, onesrow16[:], bproj_e16[:], start=False, stop=True
                    )
                    out_t = rotp.tile([128, C], F32, tag="out", name="out")
                    nc.vector.tensor_tensor(
                        out_t[:], y_ps, xst[:, nc8 * C : nc8 * C + C], op=ALU.add
                    )
                    nc.sync.dma_start(yt[sl, :], out_t[:])

                # strictly-needed prologue: k tile 0, q (nch 0); casts on
                # ACT (DVE is still finishing stats at this point)
                for pair in range(4):
                    emit_k_pair(0, pair, force="act")
                emit_q_both(0, force="act")

                # block (nch0, hp0): interleave v quads, q-nch1, k tile 1
                extra0 = {cp: (lambda cp=cp: emit_v_quad(cp)) for cp in range(8)}
                extra0[8] = lambda: emit_q_both(1)
                extra0[9] = lambda: emit_k_pair(1, 0)
                extra0[11] = lambda: emit_k_pair(1, 1)
                extra0[13] = lambda: emit_k_pair(1, 2)
                extra0[15] = lambda: emit_k_pair(1, 3)
                attention_block(0, 0, extra0)
                attention_block(0, 1, {})

                # block (nch1, hp0): interleave proj of nch0 (away from the
                # block start: proj waits on the previous normalize)
                extra2 = {2 + 2 * n: (lambda n=n: emit_proj(n)) for n in range(4)}
                attention_block(1, 0, extra2)
                attention_block(1, 1, {})
                for nc8 in range(4, 8):
                    emit_proj(nc8)

                if DEBUG_DUMPS:
                    nc.sync.dma_start(dvt_d.ap(), vt[:].bitcast(U8))
                    for t in range(2):
                        nc.sync.dma_start(dk_d.ap()[t], k_sb[t][:])
                        nc.sync.dma_start(dq_d.ap()[t], q_sb[t][:])
                        nc.sync.dma_start(du_d.ap()[t], u_sb[t][:])

    nc.compile()
    return nc


def _in_maps(inputs):
    x = np.asarray(inputs["x"], dtype=np.float32)
    gn_scale = np.asarray(inputs["gn_scale"], dtype=np.float32)
    gn_bias = np.asarray(inputs["gn_bias"], dtype=np.float32)
    w_qkv = np.asarray(inputs["w_qkv"], dtype=np.float32)
    w_proj = np.asarray(inputs["w_proj"], dtype=np.float32)
    b_proj = np.asarray(inputs["b_proj"], dtype=np.float32)

    B = x.shape[0]
    xf = x.reshape(B, C, N).astype(np.float16)
    wqkvt = np.ascontiguousarray(w_qkv.T.astype(np.float16))
    wpt = np.ascontiguousarray(w_proj.T.astype(np.float16))
    gnw = np.ascontiguousarray(gn_scale.reshape(2, 128, 1))
    gnb = np.ascontiguousarray(gn_bias.reshape(2, 128, 1))
    bproj = np.ascontiguousarray(b_proj.reshape(1, C))
    gmap = np.zeros((128, 16), dtype=np.float32)
    gmap[np.arange(128), np.arange(128) // GS] = 1.0
    gmapt = np.ascontiguousarray(gmap.T)
    onescol = np.ones((128, 1), dtype=np.float32)
    onesrow = np.ones((1, 128), dtype=np.float32)

    maps = []
    for core in range(8):
        b, s = core // 4, core % 4
        # rotate so this core's query slice sits at columns 0:1024
        # (key order is attention-invariant; k/v use the same order)
        xr = np.ascontiguousarray(np.roll(xf[b], -s * NS, axis=1))
        maps.append(
            {
                "xb": xr,
                "xst": np.ascontiguousarray(xr[:, 0:NS].T),
                "wqkvt": wqkvt,
                "wpt": wpt,
                "gnw": gnw,
                "gnb": gnb,
                "bproj": bproj,
                "gmap": gmap,
                "gmapt": gmapt,
                "onescol": onescol,
                "onesrow": onesrow,
            }
        )
    return maps


def _run(inputs, trace=False):
    if "nc" not in _cached:
        _cached["nc"] = _build()
    nc = _cached["nc"]
    maps = _in_maps(inputs)
    res = None
    for attempt in range(4):
        try:
            res = bass_utils.run_bass_kernel_spmd(
                nc, maps, core_ids=list(range(8)), trace=trace
            )
            break
        except Exception:
            if attempt == 3:
                raise
            time.sleep(10.0 * (attempt + 1))
    outs = np.stack([res.results[c]["yt"] for c in range(8)])  # [8, NS, C]
    y = outs.reshape(2, 4 * NS, C).transpose(0, 2, 1).reshape(2, C, 64, 64)
    return np.ascontiguousarray(y.astype(np.float32)), res


def kernel(**inputs):
    y, _ = _run(inputs, trace=False)
    return y
